# revision 1
# baseline (speedup 1.0000x reference)
"""Trainium2 Bass kernel for nn_Convolution_77111842832763.

3D conv 5x5x5 SAME, 64->64 channels, input [2,40,40,40,64] fp32, plus an
irrep-wise linear self-connection (folded into the conv's center tap).

Strategy (8 NeuronCores, data-parallel):
  - Shard: core = batch(2) x x-chunk(4); each core computes a [10,40,40,64]
    output slab from a zero-padded [14,44,44,64] input slab (halo 2).
  - Host builds the 5^3 x 64 x 64 tensor-product kernel exactly (float64),
    folds the self-connection into the center tap, and packs per-tap weight
    blocks; weights/slabs are cast to fp16 on host (device matmuls run fp16
    with fp32 PSUM accumulation; measured matmul rel-err ~3e-4).
  - Device: input slab in SBUF channel-major [128, 14*44*44]: partitions
    0-63 = slab, 64-127 = slab shifted by one z-voxel.  This packs two
    z-adjacent taps into one K=128 matmul (dz pairs (0,1),(2,3),(4,zero)),
    so 125 taps -> 75 matmuls per output tile.  The 75 units alternate
    between PE column groups 0-63/64-127 (2x column tiling) accumulating
    into psum[0:64] / psum[64:128]; the two partial sums are DMA'd out
    separately and added on host.
  - Output tile = one x-plane quarter: 10 y-rows x 40 z = 400 voxels
    (moving free dim 400, one PSUM bank).  40 tiles per core.
"""

import functools
import json
import math
from contextlib import ExitStack

import numpy as np

import concourse.bass as bass
import concourse.mybir as mybir
import concourse.tile as tile
import concourse.tile_sem_assignment as tsa
from concourse.bass_utils import run_bass_kernel_spmd

# All SWDGE DMAs on one Tile bookkeeping lane: the walrus build in this
# container allows at most one sync wait per instruction, and the tail drain
# waits once per DMA lane used.  One lane -> one wait.  (A single DMA already
# spreads across all 16 SDMA engines, so this does not limit DMA bandwidth.)
tsa.NUM_SWDGE_GLOBAL_SEMS = 1

MUL = 16
DIM = 64
NB = 8
PX, PY, PZ = 14, 44, 44          # padded slab dims
PLANE = PY * PZ                   # 1936
SLAB = PX * PLANE                 # 27104
SLAB_PAD = SLAB + 2               # +2 tail zeros so the z+1 view stays in-bounds
UNITS = [(dx, dy, g) for dx in range(5) for dy in range(5) for g in range(3)]
NU = len(UNITS)                   # 75


def _split_sync_waits_json(raw: bytes) -> bytes:
    """Hoist all but the last sync wait of each instruction onto preceding
    same-engine EventSemaphore instructions (engines execute in order, so
    this is semantically identical)."""
    m = json.loads(raw)
    ctr = 0
    for fn in m.get("functions", []):
        for blk in fn.get("blocks", []):
            out = []
            for inst in blk.get("instructions", []):
                si = inst.get("sync_info")
                ow = (si or {}).get("on_wait") or []
                if len(ow) > 1:
                    for w in ow[:-1]:
                        ctr += 1
                        out.append({
                            "debug": inst.get("debug", 0),
                            "engine": inst["engine"],
                            "ins": [],
                            "outs": [],
                            "name": f"SWX-{ctr}",
                            "opcode": "EventSemaphore",
                            "sync_info": {"on_update": [], "on_wait": [w]},
                        })
                    si["on_wait"] = [ow[-1]]
                out.append(inst)
            blk["instructions"] = out
    return json.dumps(m).encode()


def _build_tp_kernel(linear_weight: np.ndarray, weight: np.ndarray) -> np.ndarray:
    """Mirror reference.py's CG kernel construction in float64.
    Returns k[5,5,5,64,64] ([dx,dy,dz,in,out]) with the self-connection
    folded into the center tap."""
    lw = linear_weight.astype(np.float64)
    w8 = weight.astype(np.float64)
    ax = np.arange(-2.0, 3.0)
    gx, gy, gz = np.meshgrid(ax, ax, ax, indexing="ij")
    lattice = np.stack([gx, gy, gz], axis=-1)            # [5,5,5,3]
    rad = np.linalg.norm(lattice, axis=-1)
    values = np.linspace(0.0, 2.5, NB + 2)[1:-1]
    step = 2.5 / (NB + 1)
    diff = (rad[..., None] - values) / step
    den = np.maximum(1.0 - diff * diff, 1e-9)
    emb = np.where(np.abs(diff) < 1.0, 1.14136 * np.exp(2.0 - 1.0 / den), 0.0)
    n = rad[..., None]
    unit = np.where(n > 0, lattice / np.where(n > 0, n, 1.0), 0.0)
    sh = np.concatenate([np.ones((5, 5, 5, 1)), math.sqrt(3.0) * unit], -1)
    L = 125
    w = (emb.reshape(L, NB) @ w8) / float(L)             # [125, 1024]
    W = w.reshape(L, 4, MUL, MUL)
    shf = sh.reshape(L, 4)
    y0, y1 = shf[:, 0], shf[:, 1:4]
    a = 1.0 / math.sqrt(2.0 * MUL)
    eye3 = np.eye(3)
    Rss = a * W[:, 0] * y0[:, None, None]
    Rsv = a * np.einsum("luw,lm->luwm", W[:, 1], y1).reshape(L, MUL, 3 * MUL)
    Rvv = a * np.einsum("luw,l,mn->lumwn", W[:, 2], y0, eye3).reshape(L, 3 * MUL, 3 * MUL)
    Rvs = (a / math.sqrt(3.0)) * np.einsum("luw,lm->lumw", W[:, 3], y1).reshape(L, 3 * MUL, MUL)
    k = np.concatenate(
        [np.concatenate([Rss, Rsv], -1), np.concatenate([Rvs, Rvv], -1)], 1
    ).reshape(5, 5, 5, DIM, DIM)
    # self-connection: irrep-wise linear, folded into center tap
    Wl = lw.reshape(2, MUL, MUL) / math.sqrt(MUL)
    sc = np.zeros((DIM, DIM))
    sc[:MUL, :MUL] = Wl[0]
    for m in range(3):
        idx = MUL + np.arange(MUL) * 3 + m
        sc[np.ix_(idx, idx)] = Wl[1]
    k = k.copy()
    k[2, 2, 2] += sc
    return k


def _pack_weights(k: np.ndarray) -> np.ndarray:
    """[128, 75*64] fp16: rows 0-63 = tap dz, rows 64-127 = tap dz+1 (zeros
    for the unpaired dz=4 group)."""
    Wp = np.zeros((128, NU * DIM), np.float64)
    for ui, (dx, dy, g) in enumerate(UNITS):
        dz = 2 * g
        Wp[0:64, ui * DIM:(ui + 1) * DIM] = k[dx, dy, dz]
        if dz + 1 < 5:
            Wp[64:128, ui * DIM:(ui + 1) * DIM] = k[dx, dy, dz + 1]
    return Wp.astype(np.float16)


def _build_slab(xb: np.ndarray, cx: int) -> np.ndarray:
    """Channel-major zero-padded fp16 slab [64, SLAB_PAD] for x-chunk cx of
    batch-slice xb [40,40,40,64]."""
    pad = np.zeros((PX, PY, PZ, DIM), np.float32)
    x0 = cx * 10 - 2
    lo, hi = max(0, x0), min(40, x0 + PX)
    pad[lo - x0:hi - x0, 2:42, 2:42, :] = xb[lo:hi]
    xs = np.ascontiguousarray(pad.transpose(3, 0, 1, 2)).reshape(DIM, SLAB)
    out = np.zeros((DIM, SLAB_PAD), np.float16)
    out[:, :SLAB] = xs.astype(np.float16)
    return out


def _build_program():
    nc = bass.Bass("TRN2", target_bir_lowering=False, debug=False)
    xs_d = nc.dram_tensor("xs", [DIM, SLAB_PAD], mybir.dt.float16, kind="ExternalInput")
    wt_d = nc.dram_tensor("wt", [128, NU * DIM], mybir.dt.float16, kind="ExternalInput")
    y_d = nc.dram_tensor("y", [128, 16000], mybir.dt.float32, kind="ExternalOutput")

    with tile.TileContext(nc) as tc:
        with ExitStack() as ctx:
            wpool = ctx.enter_context(tc.tile_pool(name="wts", bufs=1))
            ppool = ctx.enter_context(tc.tile_pool(name="planes", bufs=1))
            spool = ctx.enter_context(tc.tile_pool(name="stage", bufs=3))
            qpool = ctx.enter_context(tc.tile_pool(name="psum", bufs=8, space="PSUM"))

            wt_sb = wpool.tile([128, NU * DIM], mybir.dt.float16)
            nc.gpsimd.dma_start(wt_sb[:], wt_d.ap())

            planes = []
            for i in range(PX):
                pt = ppool.tile([128, PLANE], mybir.dt.float16, name=f"plane{i}",
                                tag=f"plane{i}")
                nc.gpsimd.dma_start(pt[0:64, :], xs_d.ap()[:, i * PLANE:(i + 1) * PLANE])
                nc.gpsimd.dma_start(pt[64:128, :], xs_d.ap()[:, i * PLANE + 1:(i + 1) * PLANE + 1])
                planes.append(pt)

            for px in range(10):
                stage = spool.tile([128, 1600], mybir.dt.float32, name="stage", tag="stage")
                for ty in range(4):
                    ps = qpool.tile([128, 400], mybir.dt.float32, name="ps", tag="ps")
                    first = [True, True]
                    for ui, (dx, dy, g) in enumerate(UNITS):
                        grp = ui % 2
                        pl3 = planes[px + dx][:].rearrange("p (y z) -> p y z", y=PY)
                        yb = ty * 10 + dy
                        rhs = pl3[:, yb:yb + 10, 2 * g:2 * g + 40]
                        nc.tensor.matmul(
                            ps[grp * 64:(grp + 1) * 64, :],
                            wt_sb[:, ui * DIM:(ui + 1) * DIM],
                            rhs,
                            start=first[grp],
                            stop=(ui >= NU - 2),
                            tile_position=(0, grp * 64),
                        )
                        first[grp] = False
                    nc.vector.tensor_copy(stage[:, ty * 400:(ty + 1) * 400], ps[:])
                nc.gpsimd.dma_start(y_d.ap()[:, px * 1600:(px + 1) * 1600], stage[:])

    orig = nc.to_json_bytes
    nc.to_json_bytes = functools.wraps(orig)(lambda: _split_sync_waits_json(orig()))
    return nc


def kernel(x, linear_weight, weight, _trace=False):
    x = np.asarray(x, np.float32)
    k = _build_tp_kernel(np.asarray(linear_weight), np.asarray(weight))
    wt = _pack_weights(k)

    in_maps = []
    for core in range(8):
        b, cx = divmod(core, 4)
        in_maps.append({"xs": _build_slab(x[b], cx), "wt": wt})

    nc = _build_program()
    res = run_bass_kernel_spmd(nc, in_maps, core_ids=list(range(8)), trace=_trace)

    y = np.empty((2, 40, 40, 40, DIM), np.float32)
    for core in range(8):
        b, cx = divmod(core, 4)
        yc = res.results[core]["y"]
        s = (yc[:64] + yc[64:]).reshape(DIM, 10, 4, 10, 40)
        y[b, cx * 10:(cx + 1) * 10] = s.transpose(1, 2, 3, 4, 0).reshape(10, 40, 40, DIM)
    if _trace:
        kernel.last_results = res
    return y


# revision 2
# speedup vs baseline: 1.1951x; 1.1951x over previous
"""Trainium2 Bass kernel for nn_Convolution_77111842832763.

3D conv 5x5x5 SAME, 64->64 channels, input [2,40,40,40,64] fp32, plus an
irrep-wise linear self-connection (folded into the conv's center tap).

Strategy (8 NeuronCores, data-parallel):
  - Shard: core = batch(2) x x-chunk(4); each core computes a [10,40,40,64]
    output slab from a zero-padded [14,44,44,64] input slab (halo 2).
  - Host builds the 5^3 x 64 x 64 tensor-product kernel exactly (float64),
    folds the self-connection into the center tap, and packs per-tap weight
    blocks; weights/slabs are cast to fp16 on host (device matmuls run fp16
    with fp32 PSUM accumulation; measured end-to-end rel-err ~2.9e-4).
  - Device: TWO channel-major slab copies in SBUF [128, 14*44*44]:
      slab_z: partitions 0-63 = slab, 64-127 = slab shifted +1 z-voxel
      slab_y: partitions 0-63 = slab, 64-127 = slab shifted +1 y-row
    K=128 packing: the 125 taps become 65 matmul units per output tile:
      50 z-pair units  (dx,dy, dz in {(0,1),(2,3)})        -> slab_z
      15 y-pair units  (dx, dz=4 slice, dy in {(0,1),(2,3),(4,zero)}) -> slab_y
  - Units alternate between PE column groups 0-63/64-127 (2x column
    tiling) accumulating into psum[0:64]/psum[64:128]; the two partial
    sums are DMA'd out separately and added on host.
  - Output tile = one x-plane quarter: 10 y-rows x 40 z = 400 voxels
    (moving free dim 400, one PSUM bank).  40 tiles per core.
  - DMA: input planes on the SP HWDGE ring, weights/outputs on the ACT
    HWDGE ring (parallel rings).  A JSON post-pass splits multi-wait
    instructions (this walrus build allows one sync wait per instruction).
"""

import functools
import json
import math
from contextlib import ExitStack

import numpy as np

import concourse.bass as bass
import concourse.mybir as mybir
import concourse.tile as tile
from concourse.bass_utils import run_bass_kernel_spmd

MUL = 16
DIM = 64
NB = 8
PX, PY, PZ = 14, 44, 44          # padded slab dims
PLANE = PY * PZ                   # 1936
SLAB = PX * PLANE                 # 27104
SLAB_PAD = SLAB + 48              # tail zeros so the +1z and +44y views stay in-bounds
# unit list: ("z", dx, dy, zg) -> taps (dx,dy,2*zg)+(dx,dy,2*zg+1) via slab_z
#            ("y", dx, yg)     -> taps (dx,2*yg,4)+(dx,2*yg+1,4)   via slab_y
UNITS = [("z", dx, dy, zg) for dx in range(5) for dy in range(5) for zg in range(2)]
UNITS += [("y", dx, yg, 0) for dx in range(5) for yg in range(3)]
NU = len(UNITS)                   # 65


def _split_sync_waits_json(raw: bytes) -> bytes:
    """Hoist all but the last sync wait of each instruction onto preceding
    same-engine EventSemaphore instructions (engines execute in order, so
    this is semantically identical)."""
    m = json.loads(raw)
    ctr = 0
    for fn in m.get("functions", []):
        for blk in fn.get("blocks", []):
            out = []
            for inst in blk.get("instructions", []):
                si = inst.get("sync_info")
                ow = (si or {}).get("on_wait") or []
                if len(ow) > 1:
                    for w in ow[:-1]:
                        ctr += 1
                        out.append({
                            "debug": inst.get("debug", 0),
                            "engine": inst["engine"],
                            "ins": [],
                            "outs": [],
                            "name": f"SWX-{ctr}",
                            "opcode": "EventSemaphore",
                            "sync_info": {"on_update": [], "on_wait": [w]},
                        })
                    si["on_wait"] = [ow[-1]]
                out.append(inst)
            blk["instructions"] = out
    return json.dumps(m).encode()


def _build_tp_kernel(linear_weight: np.ndarray, weight: np.ndarray) -> np.ndarray:
    """Mirror reference.py's CG kernel construction in float64.
    Returns k[5,5,5,64,64] ([dx,dy,dz,in,out]) with the self-connection
    folded into the center tap."""
    lw = linear_weight.astype(np.float64)
    w8 = weight.astype(np.float64)
    ax = np.arange(-2.0, 3.0)
    gx, gy, gz = np.meshgrid(ax, ax, ax, indexing="ij")
    lattice = np.stack([gx, gy, gz], axis=-1)            # [5,5,5,3]
    rad = np.linalg.norm(lattice, axis=-1)
    values = np.linspace(0.0, 2.5, NB + 2)[1:-1]
    step = 2.5 / (NB + 1)
    diff = (rad[..., None] - values) / step
    den = np.maximum(1.0 - diff * diff, 1e-9)
    emb = np.where(np.abs(diff) < 1.0, 1.14136 * np.exp(2.0 - 1.0 / den), 0.0)
    n = rad[..., None]
    unit = np.where(n > 0, lattice / np.where(n > 0, n, 1.0), 0.0)
    sh = np.concatenate([np.ones((5, 5, 5, 1)), math.sqrt(3.0) * unit], -1)
    L = 125
    w = (emb.reshape(L, NB) @ w8) / float(L)             # [125, 1024]
    W = w.reshape(L, 4, MUL, MUL)
    shf = sh.reshape(L, 4)
    y0, y1 = shf[:, 0], shf[:, 1:4]
    a = 1.0 / math.sqrt(2.0 * MUL)
    eye3 = np.eye(3)
    Rss = a * W[:, 0] * y0[:, None, None]
    Rsv = a * np.einsum("luw,lm->luwm", W[:, 1], y1).reshape(L, MUL, 3 * MUL)
    Rvv = a * np.einsum("luw,l,mn->lumwn", W[:, 2], y0, eye3).reshape(L, 3 * MUL, 3 * MUL)
    Rvs = (a / math.sqrt(3.0)) * np.einsum("luw,lm->lumw", W[:, 3], y1).reshape(L, 3 * MUL, MUL)
    k = np.concatenate(
        [np.concatenate([Rss, Rsv], -1), np.concatenate([Rvs, Rvv], -1)], 1
    ).reshape(5, 5, 5, DIM, DIM)
    # self-connection: irrep-wise linear, folded into center tap
    Wl = lw.reshape(2, MUL, MUL) / math.sqrt(MUL)
    sc = np.zeros((DIM, DIM))
    sc[:MUL, :MUL] = Wl[0]
    for m in range(3):
        idx = MUL + np.arange(MUL) * 3 + m
        sc[np.ix_(idx, idx)] = Wl[1]
    k = k.copy()
    k[2, 2, 2] += sc
    return k


def _pack_weights(k: np.ndarray) -> np.ndarray:
    """[128, 65*64] fp16 per-unit weight blocks (rows 64-127 = paired tap,
    zeros when unpaired)."""
    Wp = np.zeros((128, NU * DIM), np.float64)
    for ui, u in enumerate(UNITS):
        s = ui * DIM
        if u[0] == "z":
            _, dx, dy, zg = u
            Wp[0:64, s:s + DIM] = k[dx, dy, 2 * zg]
            Wp[64:128, s:s + DIM] = k[dx, dy, 2 * zg + 1]
        else:
            _, dx, yg, _ = u
            Wp[0:64, s:s + DIM] = k[dx, 2 * yg, 4]
            if 2 * yg + 1 < 5:
                Wp[64:128, s:s + DIM] = k[dx, 2 * yg + 1, 4]
    return Wp.astype(np.float16)


def _build_slab(xb: np.ndarray, cx: int) -> np.ndarray:
    """Channel-major zero-padded fp16 slab [64, SLAB_PAD] for x-chunk cx of
    batch-slice xb [40,40,40,64]."""
    pad = np.zeros((PX, PY, PZ, DIM), np.float32)
    x0 = cx * 10 - 2
    lo, hi = max(0, x0), min(40, x0 + PX)
    pad[lo - x0:hi - x0, 2:42, 2:42, :] = xb[lo:hi]
    xs = np.ascontiguousarray(pad.transpose(3, 0, 1, 2)).reshape(DIM, SLAB)
    out = np.zeros((DIM, SLAB_PAD), np.float16)
    out[:, :SLAB] = xs.astype(np.float16)
    return out


def _build_program():
    nc = bass.Bass("TRN2", target_bir_lowering=False, debug=False)
    xs_d = nc.dram_tensor("xs", [DIM, SLAB_PAD], mybir.dt.float16, kind="ExternalInput")
    wt_d = nc.dram_tensor("wt", [128, NU * DIM], mybir.dt.float16, kind="ExternalInput")
    y_d = nc.dram_tensor("y", [128, 16000], mybir.dt.float32, kind="ExternalOutput")

    with tile.TileContext(nc) as tc:
        with ExitStack() as ctx:
            wpool = ctx.enter_context(tc.tile_pool(name="wts", bufs=1))
            ppool = ctx.enter_context(tc.tile_pool(name="planes", bufs=1))
            spool = ctx.enter_context(tc.tile_pool(name="stage", bufs=4))
            qpool = ctx.enter_context(tc.tile_pool(name="psum", bufs=8, space="PSUM"))

            wt_sb = wpool.tile([128, NU * DIM], mybir.dt.float16)
            nc.scalar.dma_start(wt_sb[:], wt_d.ap())

            pz_planes, py_planes = [], []
            for i in range(PX):
                o = i * PLANE
                tz = ppool.tile([128, PLANE], mybir.dt.float16, name=f"pz{i}", tag=f"pz{i}")
                nc.sync.dma_start(tz[0:64, :], xs_d.ap()[:, o:o + PLANE])
                nc.sync.dma_start(tz[64:128, :], xs_d.ap()[:, o + 1:o + PLANE + 1])
                pz_planes.append(tz)
                ty_ = ppool.tile([128, PLANE], mybir.dt.float16, name=f"py{i}", tag=f"py{i}")
                nc.scalar.dma_start(ty_[0:64, :], xs_d.ap()[:, o:o + PLANE])
                nc.scalar.dma_start(ty_[64:128, :], xs_d.ap()[:, o + PY:o + PLANE + PY])
                py_planes.append(ty_)

            for px in range(10):
                stage = spool.tile([128, 1600], mybir.dt.float32, name="stage", tag="stage")
                for ty in range(4):
                    ps = qpool.tile([128, 400], mybir.dt.float32, name="ps", tag="ps")
                    first = [True, True]
                    for ui, u in enumerate(UNITS):
                        grp = ui % 2
                        if u[0] == "z":
                            _, dx, dy, zg = u
                            src, yb, zo = pz_planes[px + dx], ty * 10 + dy, 2 * zg
                        else:
                            _, dx, yg, _ = u
                            src, yb, zo = py_planes[px + dx], ty * 10 + 2 * yg, 4
                        pl3 = src[:].rearrange("p (y z) -> p y z", y=PY)
                        rhs = pl3[:, yb:yb + 10, zo:zo + 40]
                        nc.tensor.matmul(
                            ps[grp * 64:(grp + 1) * 64, :],
                            wt_sb[:, ui * DIM:(ui + 1) * DIM],
                            rhs,
                            start=first[grp],
                            stop=(ui >= NU - 2),
                            tile_position=(0, grp * 64),
                        )
                        first[grp] = False
                    nc.vector.tensor_copy(stage[:, ty * 400:(ty + 1) * 400], ps[:])
                nc.scalar.dma_start(y_d.ap()[:, px * 1600:(px + 1) * 1600], stage[:])

    orig = nc.to_json_bytes
    nc.to_json_bytes = functools.wraps(orig)(lambda: _split_sync_waits_json(orig()))
    return nc


def kernel(x, linear_weight, weight, _trace=False):
    x = np.asarray(x, np.float32)
    k = _build_tp_kernel(np.asarray(linear_weight), np.asarray(weight))
    wt = _pack_weights(k)

    in_maps = []
    for core in range(8):
        b, cx = divmod(core, 4)
        in_maps.append({"xs": _build_slab(x[b], cx), "wt": wt})

    nc = _build_program()
    res = run_bass_kernel_spmd(nc, in_maps, core_ids=list(range(8)), trace=_trace)

    y = np.empty((2, 40, 40, 40, DIM), np.float32)
    for core in range(8):
        b, cx = divmod(core, 4)
        yc = res.results[core]["y"]
        s = (yc[:64] + yc[64:]).reshape(DIM, 10, 4, 10, 40)
        y[b, cx * 10:(cx + 1) * 10] = s.transpose(1, 2, 3, 4, 0).reshape(10, 40, 40, DIM)
    if _trace:
        kernel.last_results = res
    return y
